# revision 1
# baseline (speedup 1.0000x reference)
"""Trainium2 Bass kernel for ColorQuantization (soft VQ onto 4 pure colors).

Math derivation (exact rewrite of the reference):
  PURE_COLORS rows all have squared norm 3, so in
      softmax(-(|x|^2 + |c_j|^2 - 2 x.c_j)/T)
  the |x|^2 + 3 terms are constant across j and cancel. With T = 0.1 the
  weights reduce to softmax_j(20 * x.c_j). Subtracting the j=0 logit
  (colors are (-1,-1,-1),(1,-1,-1),(-1,1,-1),(-1,-1,1)):
      weights = softmax([0, 40r, 40g, 40b])
  and the output channels are
      out_r = -w0 + w1 - w2 - w3 = 2*w1 - 1   (sum w = 1)
      out_g = 2*w2 - 1,  out_b = 2*w3 - 1.
  So per pixel with e_c = exp(40*x_c), S = 1 + e1 + e2 + e3:
      out_c = 2*e_c/S - 1.
  40*x_c is in (-40, 40) so exp() never overflows fp32; no max-subtraction
  needed.

Sharding: batch dim 32 split across 8 cores (4 images per core), palette
math is hardcoded. Each image's R/G/B planes are [128, 2048] fp32 tiles.
"""

import contextlib

import numpy as np

import concourse.bacc as bacc
import concourse.mybir as mybir
from concourse.tile import TileContext
from concourse import bass_utils

N_CORES = 8
B, C, H, W = 32, 3, 512, 512
B_PER = B // N_CORES          # 4 images per core
P = 128                       # SBUF partitions
F = (H * W) // P              # 2048 free elems per partition per plane

F32 = mybir.dt.float32
Alu = mybir.AluOpType
Act = mybir.ActivationFunctionType

_BUILT = None


def _build(reps: int = 1, *, store_on_scalar: bool = False, chunk: int = F,
           rebalance: bool = False, io_bufs: int = 2, wk_bufs: int = 2,
           store_engine: str | None = None, e2_affine: str = "gpsimd"):
    nc = bacc.Bacc(trn_type="TRN2")
    x = nc.dram_tensor("x", [B_PER, C, H, W], F32, kind="ExternalInput")
    out = nc.dram_tensor("out", [B_PER, C, H, W], F32, kind="ExternalOutput")

    # plane i = (image b, channel c): [128, 2048], contiguous per partition
    xp = x.rearrange("b c (p r) w -> (b c) p (r w)", p=P)
    op = out.rearrange("b c (p r) w -> (b c) p (r w)", p=P)

    with TileContext(nc) as tc:
        with (
            tc.tile_pool(name="io", bufs=io_bufs) as io,
            tc.tile_pool(name="work", bufs=wk_bufs) as wk,
        ):
            loop_cm = tc.For_i(0, reps, 1) if reps > 1 else contextlib.nullcontext()
            with loop_cm:
                _emit_body(nc, io, wk, xp, op,
                           store_on_scalar=store_on_scalar, chunk=chunk,
                           rebalance=rebalance, store_engine=store_engine,
                           e2_affine=e2_affine)

    nc.compile()
    return nc


def _build_fused(reps: int = 1, *, imgs_per_tile: int = 1, io_bufs: int = 2,
                 store_engine: str = "sync", rebalance: bool = False,
                 exp_split: int = 1, e2_affine: str = "gpsimd"):
    """One strided DMA per image-group: tile [128, G*3*2048]; exp in place;
    per-image softmax math on slices; single store per group."""
    G = imgs_per_tile
    nc = bacc.Bacc(trn_type="TRN2")
    x = nc.dram_tensor("x", [B_PER, C, H, W], F32, kind="ExternalInput")
    out = nc.dram_tensor("out", [B_PER, C, H, W], F32, kind="ExternalOutput")

    # group g -> [128, G, 3, F]; per partition: G*3 runs of F contiguous elems
    xg = x.rearrange("(a g) c (p r) w -> a p g c (r w)", g=G, p=P)
    og = out.rearrange("(a g) c (p r) w -> a p g c (r w)", g=G, p=P)
    store_eng = {"sync": nc.sync, "scalar": nc.scalar, "gpsimd": nc.gpsimd}[store_engine]

    with TileContext(nc) as tc:
        with (
            tc.tile_pool(name="io", bufs=io_bufs) as io,
            tc.tile_pool(name="work", bufs=2) as wk,
        ):
            loop_cm = tc.For_i(0, reps, 1) if reps > 1 else contextlib.nullcontext()
            with loop_cm:
                for a in range(B_PER // G):
                    X = io.tile([P, G * 3 * F], F32, tag="X")
                    X4 = X.rearrange("p (g c f) -> p g c f", g=G, c=3)
                    nc.sync.dma_start(out=X4, in_=xg[a])
                    # exp over the whole group tile, in place
                    if exp_split == 1:
                        nc.scalar.activation(X, X, Act.Exp, bias=0.0, scale=40.0)
                    else:
                        w = G * 3 * F // exp_split
                        for k in range(exp_split):
                            ksl = slice(k * w, (k + 1) * w)
                            nc.scalar.activation(X[:, ksl], X[:, ksl], Act.Exp,
                                                 bias=0.0, scale=40.0)
                    for g in range(G):
                        base = g * 3 * F
                        e1 = X[:, base : base + F]
                        e2 = X[:, base + F : base + 2 * F]
                        e3 = X[:, base + 2 * F : base + 3 * F]
                        s = wk.tile([P, F], F32, tag="s")
                        nc.vector.scalar_tensor_tensor(
                            out=s, in0=e1, scalar=1.0, in1=e2, op0=Alu.add, op1=Alu.add
                        )
                        nc.vector.tensor_add(s, s, e3)
                        nc.vector.reciprocal_approx_fast(out=s, in_=s)

                        nc.vector.tensor_mul(e1, e1, s)
                        if rebalance:
                            nc.gpsimd.tensor_mul(e2, e2, s)
                        else:
                            nc.vector.tensor_mul(e2, e2, s)
                        nc.vector.tensor_mul(e3, e3, s)

                        nc.vector.tensor_scalar(e1, e1, 2.0, -1.0, Alu.mult, Alu.add)
                        if rebalance or e2_affine == "vector":
                            nc.vector.tensor_scalar(e2, e2, 2.0, -1.0, Alu.mult, Alu.add)
                        elif e2_affine == "scalar":
                            nc.scalar.activation(e2, e2, Act.Copy, bias=-1.0, scale=2.0)
                        else:
                            nc.gpsimd.tensor_scalar(e2, e2, 2.0, -1.0, Alu.mult, Alu.add)
                        nc.scalar.activation(e3, e3, Act.Copy, bias=-1.0, scale=2.0)
                    store_eng.dma_start(out=og[a], in_=X4)

    nc.compile()
    return nc


def _emit_body(nc, io, wk, xp, op, *, store_on_scalar, chunk, rebalance,
               store_engine=None, e2_affine="gpsimd"):
    if store_engine is None:
        store_engine = "scalar" if store_on_scalar else "sync"
    store_eng = {"sync": nc.sync, "scalar": nc.scalar, "gpsimd": nc.gpsimd,
                 "vector": nc.vector}[store_engine]
    n_chunks = F // chunk
    for b in range(B_PER):
        for ci in range(n_chunks):
            sl = slice(ci * chunk, (ci + 1) * chunk)
            r = io.tile([P, chunk], F32, tag="r")
            g = io.tile([P, chunk], F32, tag="g")
            bl = io.tile([P, chunk], F32, tag="bl")
            nc.sync.dma_start(out=r, in_=xp[3 * b + 0][:, sl])
            nc.sync.dma_start(out=g, in_=xp[3 * b + 1][:, sl])
            nc.sync.dma_start(out=bl, in_=xp[3 * b + 2][:, sl])

            e1 = wk.tile([P, chunk], F32, tag="e1")
            e2 = wk.tile([P, chunk], F32, tag="e2")
            e3 = wk.tile([P, chunk], F32, tag="e3")
            nc.scalar.activation(e1, r, Act.Exp, bias=0.0, scale=40.0)
            nc.scalar.activation(e2, g, Act.Exp, bias=0.0, scale=40.0)
            nc.scalar.activation(e3, bl, Act.Exp, bias=0.0, scale=40.0)

            # s = 1 + e1 + e2 + e3;  v = 1/s  (in place)
            s = wk.tile([P, chunk], F32, tag="s")
            nc.vector.scalar_tensor_tensor(
                out=s, in0=e1, scalar=1.0, in1=e2, op0=Alu.add, op1=Alu.add
            )
            nc.vector.tensor_add(s, s, e3)
            nc.vector.reciprocal_approx_fast(out=s, in_=s)

            # q_c = e_c * v (in place on e_c), then out_c = 2*q_c - 1,
            # spread across engines
            nc.vector.tensor_mul(e1, e1, s)
            if rebalance:
                nc.gpsimd.tensor_mul(e2, e2, s)
            else:
                nc.vector.tensor_mul(e2, e2, s)
            nc.vector.tensor_mul(e3, e3, s)

            nc.vector.tensor_scalar(e1, e1, 2.0, -1.0, Alu.mult, Alu.add)
            if rebalance:
                nc.vector.tensor_scalar(e2, e2, 2.0, -1.0, Alu.mult, Alu.add)
            elif e2_affine == "vector":
                nc.vector.tensor_scalar(e2, e2, 2.0, -1.0, Alu.mult, Alu.add)
            elif e2_affine == "scalar":
                nc.scalar.activation(e2, e2, Act.Copy, bias=-1.0, scale=2.0)
            else:
                nc.gpsimd.tensor_scalar(e2, e2, 2.0, -1.0, Alu.mult, Alu.add)
            nc.scalar.activation(e3, e3, Act.Copy, bias=-1.0, scale=2.0)

            store_eng.dma_start(out=op[3 * b + 0][:, sl], in_=e1)
            store_eng.dma_start(out=op[3 * b + 1][:, sl], in_=e2)
            store_eng.dma_start(out=op[3 * b + 2][:, sl], in_=e3)


def _get_built():
    global _BUILT
    if _BUILT is None:
        _BUILT = _build()
    return _BUILT


def _run(x: np.ndarray, trace: bool = False):
    nc = _get_built()
    x = np.ascontiguousarray(np.asarray(x, dtype=np.float32))
    assert x.shape == (B, C, H, W), x.shape
    in_maps = [{"x": x[i * B_PER : (i + 1) * B_PER]} for i in range(N_CORES)]
    res = bass_utils.run_bass_kernel_spmd(
        nc, in_maps, core_ids=list(range(N_CORES)), trace=trace
    )
    out = np.concatenate([r["out"] for r in res.results], axis=0)
    return out, res


def kernel(**inputs) -> np.ndarray:
    out, _ = _run(inputs["x"], trace=False)
    return out


def kernel_profiled(**inputs):
    """Returns (output, BassKernelResults) with HW trace enabled.
    Falls back to trace=False when the axon NTFF profiling hook is
    unavailable in this container."""
    try:
        return _run(inputs["x"], trace=True)
    except (ModuleNotFoundError, ImportError):
        return _run(inputs["x"], trace=False)



# revision 2
# speedup vs baseline: 1.0678x; 1.0678x over previous
"""Trainium2 Bass kernel for ColorQuantization (soft VQ onto 4 pure colors).

Math derivation (exact rewrite of the reference):
  PURE_COLORS rows all have squared norm 3, so in
      softmax(-(|x|^2 + |c_j|^2 - 2 x.c_j)/T)
  the |x|^2 + 3 terms are constant across j and cancel. With T = 0.1 the
  weights reduce to softmax_j(20 * x.c_j). Subtracting the j=0 logit
  (colors are (-1,-1,-1),(1,-1,-1),(-1,1,-1),(-1,-1,1)):
      weights = softmax([0, 40r, 40g, 40b])
  and the output channels are out_c = 2*w_c - 1 (c in {r,g,b}).
  So per pixel with e_c = exp(40*x_c), S = 1 + e1 + e2 + e3:
      out_c = 2*e_c/S - 1.
  40*x_c is in (-40, 40) so exp() never overflows fp32.

Sharding: batch dim 32 split across 8 cores (4 images per core).

Kernel structure ("v7 load-ahead", HW-measured via reps-loop slope):
  - per (image, channel) plane tiles [128, 2048] fp32 (1MB DMAs)
  - ALL 12 plane loads are issued first, and the stores are queued
    behind them on the SAME sync HWDGE ring: FIFO order makes the DMA
    engines drain every load before any store, so the last image's
    compute overlaps the store phase instead of becoming an exposed
    ~9us tail (HBM-per-core ~358 GB/s caps 24MB of traffic at ~70us;
    the old interleaved order measured 87us, this order ~77us)
  - the io pool holds all 12 plane tiles (12MB); mul/affine results are
    written back into the io tiles so the wk pool stays at 2 bufs (8MB)
  - exp on ACT; sums/recip/muls on DVE; the three final 2q-1 affines
    spread over DVE/GPSIMD/ACT. GPSIMD gets only 1-input ops (2-input
    GPSIMD ops contend with DVE's 2-port SBUF mode; measured +20us)
"""

import contextlib

import numpy as np

import concourse.bacc as bacc
import concourse.mybir as mybir
from concourse.tile import TileContext
from concourse import bass_utils

N_CORES = 8
B, C, H, W = 32, 3, 512, 512
B_PER = B // N_CORES          # 4 images per core
P = 128                       # SBUF partitions
F = (H * W) // P              # 2048 free elems per partition per plane

F32 = mybir.dt.float32
BF16 = mybir.dt.bfloat16
Alu = mybir.AluOpType
Act = mybir.ActivationFunctionType

USE_BF16 = False   # bf16 TT measured SLOWER on HW (115us) + rel_err 1.2e-2
TS1 = "vector"     # ts1 on GPS measured slower (83.3us) -- GPS op latency
                   # sits on the store-ready chain; DVE keeps it tight

_BUILT = None


def _build(reps: int = 1, *, use_bf16: bool = USE_BF16, ts1: str = TS1,
           wk_bufs: int = 2, store_engine: str = "sync"):
    nc = bacc.Bacc(trn_type="TRN2")
    x = nc.dram_tensor("x", [B_PER, C, H, W], F32, kind="ExternalInput")
    out = nc.dram_tensor("out", [B_PER, C, H, W], F32, kind="ExternalOutput")

    # plane (b, c): [128, 2048], 8KB contiguous per partition
    xp = x.rearrange("b c (p r) w -> (b c) p (r w)", p=P)
    op = out.rearrange("b c (p r) w -> (b c) p (r w)", p=P)
    st = {"sync": nc.sync, "scalar": nc.scalar, "gpsimd": nc.gpsimd}[store_engine]
    edt = BF16 if use_bf16 else F32

    with TileContext(nc) as tc:
        with (
            tc.tile_pool(name="io", bufs=B_PER) as io,
            tc.tile_pool(name="work", bufs=wk_bufs) as wk,
        ):
            loop_cm = tc.For_i(0, reps, 1) if reps > 1 else contextlib.nullcontext()
            with loop_cm:
                planes = []
                for b in range(B_PER):
                    r = io.tile([P, F], F32, tag="r")
                    g = io.tile([P, F], F32, tag="g")
                    bl = io.tile([P, F], F32, tag="bl")
                    nc.sync.dma_start(out=r, in_=xp[3 * b + 0])
                    nc.sync.dma_start(out=g, in_=xp[3 * b + 1])
                    nc.sync.dma_start(out=bl, in_=xp[3 * b + 2])
                    planes.append((r, g, bl))

                for b in range(B_PER):
                    r, g, bl = planes[b]
                    e1 = wk.tile([P, F], edt, tag="e1")
                    e2 = wk.tile([P, F], edt, tag="e2")
                    e3 = wk.tile([P, F], edt, tag="e3")
                    nc.scalar.activation(e1, r, Act.Exp, bias=0.0, scale=40.0)
                    nc.scalar.activation(e2, g, Act.Exp, bias=0.0, scale=40.0)
                    nc.scalar.activation(e3, bl, Act.Exp, bias=0.0, scale=40.0)

                    s = wk.tile([P, F], F32, tag="s")
                    if use_bf16:
                        # t = (e1+1)+e2 in bf16 (2x), s = t+e3 -> fp32,
                        # recip fp32 in place, rb = bf16(1/S) for the muls
                        t = wk.tile([P, F], BF16, tag="t")
                        nc.vector.scalar_tensor_tensor(
                            out=t, in0=e1, scalar=1.0, in1=e2,
                            op0=Alu.add, op1=Alu.add)
                        nc.vector.tensor_add(s, t, e3)
                        nc.vector.reciprocal_approx_fast(out=s, in_=s)
                        rb = wk.tile([P, F], BF16, tag="rb")
                        nc.gpsimd.tensor_copy(rb, s)
                        mul_in = rb
                    else:
                        nc.vector.scalar_tensor_tensor(
                            out=s, in0=e1, scalar=1.0, in1=e2,
                            op0=Alu.add, op1=Alu.add)
                        nc.vector.tensor_add(s, s, e3)
                        nc.vector.reciprocal_approx_fast(out=s, in_=s)
                        mul_in = s

                    # q_c = e_c / S -> fp32 back into the io tiles
                    nc.vector.tensor_mul(r, e1, mul_in)
                    nc.vector.tensor_mul(g, e2, mul_in)
                    nc.vector.tensor_mul(bl, e3, mul_in)
                    # out_c = 2*q_c - 1 in place
                    if ts1 == "vector":
                        nc.vector.tensor_scalar(r, r, 2.0, -1.0, Alu.mult, Alu.add)
                    else:
                        nc.gpsimd.tensor_scalar(r, r, 2.0, -1.0, Alu.mult, Alu.add)
                    nc.gpsimd.tensor_scalar(g, g, 2.0, -1.0, Alu.mult, Alu.add)
                    nc.scalar.activation(bl, bl, Act.Copy, bias=-1.0, scale=2.0)

                    st.dma_start(out=op[3 * b + 0], in_=r)
                    st.dma_start(out=op[3 * b + 1], in_=g)
                    st.dma_start(out=op[3 * b + 2], in_=bl)

    nc.compile()
    return nc


def _get_built():
    global _BUILT
    if _BUILT is None:
        _BUILT = _build()
    return _BUILT


def _run(x: np.ndarray, trace: bool = False):
    nc = _get_built()
    x = np.ascontiguousarray(np.asarray(x, dtype=np.float32))
    assert x.shape == (B, C, H, W), x.shape
    in_maps = [{"x": x[i * B_PER : (i + 1) * B_PER]} for i in range(N_CORES)]
    res = bass_utils.run_bass_kernel_spmd(
        nc, in_maps, core_ids=list(range(N_CORES)), trace=trace
    )
    out = np.concatenate([r["out"] for r in res.results], axis=0)
    return out, res


def kernel(**inputs) -> np.ndarray:
    out, _ = _run(inputs["x"], trace=False)
    return out


def kernel_profiled(**inputs):
    """Returns (output, BassKernelResults) with HW trace enabled.
    Falls back to trace=False when the axon NTFF profiling hook is
    unavailable in this container."""
    try:
        return _run(inputs["x"], trace=True)
    except (ModuleNotFoundError, ImportError):
        return _run(inputs["x"], trace=False)


# revision 3
# speedup vs baseline: 1.1132x; 1.0425x over previous
"""Trainium2 Bass kernel for ColorQuantization (soft VQ onto 4 pure colors).

Math derivation (exact rewrite of the reference):
  PURE_COLORS rows all have squared norm 3, so in
      softmax(-(|x|^2 + |c_j|^2 - 2 x.c_j)/T)
  the |x|^2 + 3 terms are constant across j and cancel. With T = 0.1 the
  weights reduce to softmax_j(20 * x.c_j). Subtracting the j=0 logit
  (colors are (-1,-1,-1),(1,-1,-1),(-1,1,-1),(-1,-1,1)):
      weights = softmax([0, 40r, 40g, 40b])
  and the output channels are out_c = 2*w_c - 1 (c in {r,g,b}).
  So per pixel with e_c = exp(40*x_c), S = 1 + e1 + e2 + e3:
      out_c = 2*e_c/S - 1.
  40*x_c is in (-40, 40) so exp() never overflows fp32.

Sharding: batch dim 32 split across 8 cores (4 images per core).

Kernel structure ("v8": load-ahead + per-channel interleave; HW-measured
via reps-loop slope, see test.py):
  - per (image, channel) plane tiles [128, 2048] fp32 (1MB DMAs)
  - ALL 12 plane loads are issued first, stores queued behind them on
    the SAME sync HWDGE ring: FIFO order makes the DMA engines drain
    every load before any store, so compute overlaps the store phase
    instead of becoming an exposed tail (HBM-per-core ~358 GB/s caps
    24MB of traffic at ~70us; interleaved order measured 87us)
  - the io pool holds all 12 plane tiles (12MB); mul/affine results are
    written back into the io tiles so the wk pool stays at 2 bufs (8MB)
  - per channel the emission order is mul -> affine -> store, so each
    store is ready right after its own mul; otherwise the last image's
    three stores all wait for the end of the whole DVE mul block
  - exp on ACT; sums/recip/muls and the r-channel affine on DVE; the
    g affine on GPSIMD, the b affine on ACT. GPSIMD gets only 1-input
    ops (2-input GPSIMD ops contend with DVE's 2-port SBUF mode;
    measured +20us). bf16 compute measured slower (115us) despite
    passing accuracy (1.2e-2), so fp32 it is.
"""

import contextlib

import numpy as np

import concourse.bacc as bacc
import concourse.mybir as mybir
from concourse.tile import TileContext
from concourse import bass_utils

N_CORES = 8
B, C, H, W = 32, 3, 512, 512
B_PER = B // N_CORES          # 4 images per core
P = 128                       # SBUF partitions
F = (H * W) // P              # 2048 free elems per partition per plane

F32 = mybir.dt.float32
Alu = mybir.AluOpType
Act = mybir.ActivationFunctionType

_BUILT = None


def _build(reps: int = 1, *, wk_bufs: int = 2):
    nc = bacc.Bacc(trn_type="TRN2")
    x = nc.dram_tensor("x", [B_PER, C, H, W], F32, kind="ExternalInput")
    out = nc.dram_tensor("out", [B_PER, C, H, W], F32, kind="ExternalOutput")

    # plane (b, c): [128, 2048], 8KB contiguous per partition
    xp = x.rearrange("b c (p r) w -> (b c) p (r w)", p=P)
    op = out.rearrange("b c (p r) w -> (b c) p (r w)", p=P)

    with TileContext(nc) as tc:
        with (
            tc.tile_pool(name="io", bufs=B_PER) as io,
            tc.tile_pool(name="work", bufs=wk_bufs) as wk,
        ):
            loop_cm = tc.For_i(0, reps, 1) if reps > 1 else contextlib.nullcontext()
            with loop_cm:
                planes = []
                for b in range(B_PER):
                    r = io.tile([P, F], F32, tag="r")
                    g = io.tile([P, F], F32, tag="g")
                    bl = io.tile([P, F], F32, tag="bl")
                    nc.sync.dma_start(out=r, in_=xp[3 * b + 0])
                    nc.sync.dma_start(out=g, in_=xp[3 * b + 1])
                    nc.sync.dma_start(out=bl, in_=xp[3 * b + 2])
                    planes.append((r, g, bl))

                for b in range(B_PER):
                    r, g, bl = planes[b]
                    e1 = wk.tile([P, F], F32, tag="e1")
                    e2 = wk.tile([P, F], F32, tag="e2")
                    e3 = wk.tile([P, F], F32, tag="e3")
                    nc.scalar.activation(e1, r, Act.Exp, bias=0.0, scale=40.0)
                    nc.scalar.activation(e2, g, Act.Exp, bias=0.0, scale=40.0)
                    nc.scalar.activation(e3, bl, Act.Exp, bias=0.0, scale=40.0)

                    # s = 1/(1 + e1 + e2 + e3)
                    s = wk.tile([P, F], F32, tag="s")
                    nc.vector.scalar_tensor_tensor(
                        out=s, in0=e1, scalar=1.0, in1=e2,
                        op0=Alu.add, op1=Alu.add)
                    nc.vector.tensor_add(s, s, e3)
                    nc.vector.reciprocal_approx_fast(out=s, in_=s)

                    # per channel: q_c = e_c * s into the io tile, affine
                    # 2q-1 in place, store -- readiness matches ring order
                    nc.vector.tensor_mul(r, e1, s)
                    nc.vector.tensor_scalar(r, r, 2.0, -1.0, Alu.mult, Alu.add)
                    nc.sync.dma_start(out=op[3 * b + 0], in_=r)

                    nc.vector.tensor_mul(g, e2, s)
                    nc.gpsimd.tensor_scalar(g, g, 2.0, -1.0, Alu.mult, Alu.add)
                    nc.sync.dma_start(out=op[3 * b + 1], in_=g)

                    nc.vector.tensor_mul(bl, e3, s)
                    nc.scalar.activation(bl, bl, Act.Copy, bias=-1.0, scale=2.0)
                    nc.sync.dma_start(out=op[3 * b + 2], in_=bl)

    nc.compile()
    return nc


def _get_built():
    global _BUILT
    if _BUILT is None:
        _BUILT = _build()
    return _BUILT


def _run(x: np.ndarray, trace: bool = False):
    nc = _get_built()
    x = np.ascontiguousarray(np.asarray(x, dtype=np.float32))
    assert x.shape == (B, C, H, W), x.shape
    in_maps = [{"x": x[i * B_PER : (i + 1) * B_PER]} for i in range(N_CORES)]
    res = bass_utils.run_bass_kernel_spmd(
        nc, in_maps, core_ids=list(range(N_CORES)), trace=trace
    )
    out = np.concatenate([r["out"] for r in res.results], axis=0)
    return out, res


def kernel(**inputs) -> np.ndarray:
    out, _ = _run(inputs["x"], trace=False)
    return out


def kernel_profiled(**inputs):
    """Returns (output, BassKernelResults) with HW trace enabled.
    Falls back to trace=False when the axon NTFF profiling hook is
    unavailable in this container."""
    try:
        return _run(inputs["x"], trace=True)
    except (ModuleNotFoundError, ImportError):
        return _run(inputs["x"], trace=False)
